# revision 44
# baseline (speedup 1.0000x reference)
"""Trainium2 Bass kernel for nn_MultiHeadAttention_79723182949055.

Math (per reference):
    r1 = einsum('bmp,kpd->bmkd', y, Lam_x)          # key proj
    s  = einsum('bnq,kqd->bnkd', y, Lam_y)          # query proj
    S  = einsum('bmkd,bnkd->kbmn', r1, s) + mask    # scores
    A  = softmax(S / sqrt(D), axis=m)
    w  = einsum('bmp,kpd->bmkd', y, Th_x)           # value proj
    U  = einsum('kbmn,bmkd->bnkd', A, w)            # aggregation
    out= einsum('bnkd,kqd->bnq', U, Th_y)           # out proj

Sharding: 8 cores; core c handles batch b = c//4 and heads 4*(c%4) .. +4.
Each core computes a partial out[b] (sum over its 4 heads); host sums the
4 partials per batch.

Device algorithm per core (all matmuls fp32r, full PE rate):
  - yT [P=1024, M=2048] host-pretransposed input, DMA'd interleaved with
    the per-head weight slices so phase 1 is not gated on trailing DMAs.
  - r1T[hp] [128 = 2 heads x 64 d, M] via weight-stationary matmuls.
  - std[j] [128, N]: head j's query projection s_j on the 64 partitions
    matching head j's rows in r1T[hp], ZEROS on the other 64.  Scores
    then use stationary = r1T[hp][:, m-chunk] directly (contraction over
    128 partitions at full PE rate); the zero half kills the other
    head's contribution.  No block-diagonal stationary build needed.
  - w4[m] [128, 4*65]: per-head 64 value-proj columns + a ones column
    (colsum of E rides along in the aggregation matmul for free).
  - phase 3 per (n-block nbp, head j), software-pipelined depth 3:
    scores matmul -> exp on ACT -> aggregation matmul into pu[65,1024].
    PE issue order runs scores of step m while the aggregation of step
    m-3 retires, so the PE never waits on the exp stream.
  - normalization: colsum row pu[64:65] -> DVE reciprocal_approx_fast
    -> ones-matmul broadcast to [64, 512] tiles -> DVE multiply of the
    uta slices.  ACT performs ONLY Exp all kernel long (no activation
    table reloads).
  - out[n, q] = sum over (k,d) of uta[kd, n] * thyT[kd, q], emitted
    interleaved into the next n-block's score loop so the final
    projection and output DMA hide under the exp stream.
"""

import numpy as np

import concourse.bass as bass
import concourse.bacc as bacc
import concourse.tile as tile
import concourse.mybir as mybir
from concourse.bass_utils import run_bass_kernel_spmd

F32 = mybir.dt.float32
F32R = mybir.dt.float32r

B, N, Q, K, P, D = 2, 2048, 1024, 16, 1024, 64
HPC = 4          # heads per core
NCORES = 8
INV_TEMP = 1.0 / 8.0    # 1/sqrt(D)

PCH = P // 128   # 8 p-chunks
MCH = N // 128   # 16 m-chunks
NBP = N // 1024  # 2 n-blocks of 1024
DEPTH = 5        # phase-3 software pipeline depth


def build_program(use_mask: bool):
    nc = bacc.Bacc("TRN2", target_bir_lowering=False, debug=False,
                   num_devices=NCORES)

    yT = nc.dram_tensor("yT", [P, N], F32R, kind="ExternalInput").ap()
    lamx = nc.dram_tensor("lamx", [P, HPC * D], F32R, kind="ExternalInput").ap()
    lamy = nc.dram_tensor("lamy", [P, HPC * D], F32R, kind="ExternalInput").ap()
    thx = nc.dram_tensor("thx", [P, HPC * D], F32R, kind="ExternalInput").ap()
    thyT = nc.dram_tensor("thyT", [HPC * D, Q], F32R, kind="ExternalInput").ap()
    if use_mask:
        maskd = nc.dram_tensor("mask", [N, N], F32, kind="ExternalInput").ap()
    outp = nc.dram_tensor("outp", [N, Q], F32, kind="ExternalOutput").ap()

    lp = nc.allow_low_precision(reason="fp32r matmul pipeline by design")
    lp.__enter__()
    with tile.TileContext(nc) as tc:
        with (
            tc.tile_pool(name="big8k", bufs=8) as bp,
            tc.tile_pool(name="wpool", bufs=1) as wp,
            tc.tile_pool(name="small", bufs=1) as sp,
            tc.tile_pool(name="epool", bufs=5) as ep,
            tc.tile_pool(name="opool", bufs=2) as op,
            tc.tile_pool(name="ps_s", bufs=2, space="PSUM") as pps,
            tc.tile_pool(name="ps_u", bufs=1, space="PSUM") as ppu,
            tc.tile_pool(name="ps_o", bufs=2, space="PSUM") as ppo,
        ):
            # ---- input DMA, p-interleaved so weights don't trail ----
            yt, wx, wy, wt = [], [], [], []
            for p in range(PCH):
                t = bp.tile([128, N], F32R, tag="big", name=f"yt{p}")
                nc.gpsimd.dma_start(out=t, in_=yT[p * 128:(p + 1) * 128, :])
                yt.append(t)
                tx = wp.tile([128, HPC * D], F32R, tag=f"wx{p}", name=f"wx{p}")
                nc.sync.dma_start(out=tx, in_=lamx[p * 128:(p + 1) * 128, :])
                wx.append(tx)
                ty_ = wp.tile([128, HPC * D], F32R, tag=f"wy{p}", name=f"wy{p}")
                nc.sync.dma_start(out=ty_, in_=lamy[p * 128:(p + 1) * 128, :])
                wy.append(ty_)
                tt = wp.tile([128, HPC * D], F32R, tag=f"wt{p}", name=f"wt{p}")
                nc.sync.dma_start(out=tt, in_=thx[p * 128:(p + 1) * 128, :])
                wt.append(tt)
            thyt = []
            for j2 in range(2):
                t = wp.tile([128, Q], F32R, tag=f"thyT{j2}", name=f"thyT{j2}")
                nc.sync.dma_start(out=t, in_=thyT[j2 * 128:(j2 + 1) * 128, :])
                thyt.append(t)

            onescol = sp.tile([128, 1], F32, tag="onescol", name="onescol")
            nc.vector.memset(onescol, 1.0)
            ones64 = sp.tile([65, 64], F32R, tag="ones64", name="ones64")
            nc.vector.tensor_copy(
                out=ones64, in_=onescol[0:65, 0:1].broadcast_to((65, 64)))

            # std[j]: head j's s on its 64 partitions, zeros on the other 64.
            # Zero halves are constant: memset once, during the DMA window.
            std = [bp.tile([128, N], F32R, tag="big", name=f"std{j}")
                   for j in range(HPC)]
            for j in range(HPC):
                zlo, zhi = (64, 128) if j % 2 == 0 else (0, 64)
                nc.vector.memset(std[j][zlo:zhi, :].bitcast(F32), 0.0)

            # w4[m]: [w_j (64 cols) | 1] x 4 heads; ones cols memset once.
            w4 = [sp.tile([128, HPC * 65], F32R, tag=f"w4_{m}", name=f"w4_{m}")
                  for m in range(MCH)]
            for m in range(MCH):
                nc.vector.tensor_copy(
                    out=w4[m],
                    in_=onescol[:, 0:1].broadcast_to((128, HPC * 65)))

            # ---- phase 1a: r1T per head-pair ----
            r1T = []
            for hp in range(2):
                dst = bp.tile([128, N], F32R, tag="big", name=f"r1T{hp}")
                for mbp in range(2):
                    acc = pps.tile([128, 1024], F32, tag="ps", name="acc")
                    for p in range(PCH):
                        for mh in range(2):
                            mb = mbp * 2 + mh
                            nc.tensor.matmul(
                                acc[:, mh * 512:(mh + 1) * 512],
                                wx[p][:, hp * 128:(hp + 1) * 128],
                                yt[p][:, mb * 512:(mb + 1) * 512],
                                start=(p == 0), stop=(p == PCH - 1),
                            )
                    nc.vector.tensor_copy(
                        out=dst[:, mbp * 1024:(mbp + 1) * 1024], in_=acc)
                r1T.append(dst)

            # ---- phase 1b: query projections into std halves ----
            for hp in range(2):
                for mbp in range(2):
                    acc = pps.tile([128, 1024], F32, tag="ps", name="accs")
                    for p in range(PCH):
                        for mh in range(2):
                            mb = mbp * 2 + mh
                            nc.tensor.matmul(
                                acc[:, mh * 512:(mh + 1) * 512],
                                wy[p][:, hp * 128:(hp + 1) * 128],
                                yt[p][:, mb * 512:(mb + 1) * 512],
                                start=(p == 0), stop=(p == PCH - 1),
                            )
                    sl = slice(mbp * 1024, (mbp + 1) * 1024)
                    nc.scalar.copy(out=std[2 * hp][0:64, sl], in_=acc[0:64, :])
                    nc.scalar.copy(out=std[2 * hp + 1][64:128, sl],
                                   in_=acc[64:128, :])

            # ---- phase 2: value projections into w4 ----
            for m in range(MCH):
                accw = pps.tile([128, 1024], F32, tag="ps", name="accw")
                for p in range(PCH):
                    nc.tensor.matmul(
                        accw[:, 0:HPC * D],
                        yt[p][:, m * 128:(m + 1) * 128],
                        wt[p][:, :],
                        start=(p == 0), stop=(p == PCH - 1),
                    )
                src3 = accw[:, 0:HPC * D].rearrange("p (h c) -> p h c", h=HPC)
                dst3 = w4[m].rearrange("p (h c) -> p h c", h=HPC,
                                       c=65)[:, :, 0:64]
                nc.vector.tensor_copy(out=dst3, in_=src3)

            # ---- phase 3 + interleaved phase 4 ----
            uta = [bp.tile([128, N], F32R, tag="big", name=f"uta{j2}")
                   for j2 in range(2)]
            # colsum row, its reciprocal, and the f32r-rounded copy (all at
            # partition 0: custom DVE ops misbehave at other partitions).
            # One pair's tail finishes before the next pair's tail starts,
            # so a single buffer suffices.
            csr = sp.tile([1, 1024], F32, tag="csr", name="csr")
            inv0 = sp.tile([1, 1024], F32, tag="inv0", name="inv0")
            invr = sp.tile([1, 1024], F32R, tag="invr", name="invr")

            pair_idx = [0]

            def make_tail(pu, hp, po, n0):
                """Tail of one pair as (step, thunk) items emitted a few
                steps into the NEXT pair: drains pu early (so the single
                pu accumulator frees before the next pair's first
                aggregation), then normalizes the uta slice (broadcast the
                raw colsum with a ones-matmul, reciprocal on DVE,
                multiply)."""
                def drain():
                    nc.vector.tensor_copy(out=csr, in_=pu[64:65, :])
                    nc.vector.tensor_copy(
                        out=uta[hp][po:po + 64, n0:n0 + 1024],
                        in_=pu[0:64, :])
                    nc.vector.reciprocal_approx_fast(out=inv0, in_=csr)
                    nc.vector.tensor_copy(out=invr, in_=inv0)

                def bcast():
                    for h in range(2):
                        pb = ppo.tile([64, 512], F32, tag="po", name="pb")
                        nc.tensor.matmul(
                            pb, ones64[0:1, :],
                            invr[:, h * 512:(h + 1) * 512],
                            start=True, stop=True)
                        nsl = slice(n0 + h * 512, n0 + (h + 1) * 512)
                        nc.vector.tensor_mul(
                            uta[hp][po:po + 64, nsl],
                            uta[hp][po:po + 64, nsl],
                            pb,
                        )
                return [(0, drain), (2, bcast)]

            def emit_pair(nbp, j, tail_ops, extra, extra_start):
                """One (n-block, head) pair: 16 m-steps, pipeline DEPTH.
                tail_ops: previous pair's deferred (step, thunk) items.
                extra: out-proj thunks of the previous n-block, consumed
                one per step starting at extra_start."""
                hp, po = j // 2, 64 * (j % 2)
                n0 = nbp * 1024
                e_t = {}
                pu = ppu.tile([65, 1024], F32, tag="pu", name="pu")
                for step in range(MCH + DEPTH):
                    while tail_ops and tail_ops[0][0] <= step:
                        tail_ops.pop(0)[1]()
                    if step >= DEPTH:
                        m2 = step - DEPTH
                        for h in range(2):
                            nc.tensor.matmul(
                                pu[:, h * 512:(h + 1) * 512],
                                w4[m2][:, j * 65:j * 65 + 65],
                                e_t[m2][:, h * 512:(h + 1) * 512],
                                start=(m2 == 0), stop=(m2 == MCH - 1))
                        e_t.pop(m2)
                    if step < MCH:
                        m = step
                        pst = pps.tile([128, 1024], F32, tag="ps", name="ps")
                        for h in range(2):
                            nc.tensor.matmul(
                                pst[:, h * 512:(h + 1) * 512],
                                r1T[hp][:, m * 128:(m + 1) * 128],
                                std[j][:, n0 + h * 512:n0 + (h + 1) * 512],
                                start=True, stop=True,
                            )
                        if use_mask:
                            mt = op.tile([128, 1024], F32, tag="mt", name="mt")
                            nc.gpsimd.dma_start(
                                out=mt,
                                in_=maskd[m * 128:(m + 1) * 128, n0:n0 + 1024])
                            nc.vector.tensor_add(pst, pst, mt)
                        et = ep.tile([128, 1024], F32R, tag="e", name="e")
                        e_t[m] = et
                        nc.scalar.activation(
                            out=et, in_=pst,
                            func=mybir.ActivationFunctionType.Exp,
                            scale=INV_TEMP,
                        )
                    if extra and step >= extra_start and step % 3 == 2:
                        extra.pop(0)()

                pair_idx[0] += 1
                return make_tail(pu, hp, po, n0)

            def outproj_slots(nbp, alt_pool=False):
                """Final projection for n-block nbp as a list of per-step
                thunks (interleaved into the next n-block's pairs).
                alt_pool: alternate PSUM pools (only safe when the score
                pipeline is finished — keeps the PE fed in the tail)."""
                slots = []
                for i, (nch, qb) in enumerate(
                        (nc_, qb_) for nc_ in range(nbp * 8, nbp * 8 + 8)
                        for qb_ in range(2)):
                    def mmstep(nch=nch, qb=qb, i=i):
                        if alt_pool and i % 2 == 1:
                            po_ = pps.tile([128, 1024], F32, tag="ps",
                                           name="po_")[:, 0:512]
                        else:
                            po_ = ppo.tile([128, 512], F32, tag="po",
                                           name="po_")
                        for j2 in range(2):
                            nc.tensor.matmul(
                                po_,
                                uta[j2][:, nch * 128:(nch + 1) * 128],
                                thyt[j2][:, qb * 512:(qb + 1) * 512],
                                start=(j2 == 0), stop=(j2 == 1),
                            )
                        osb = op.tile([128, 512], F32, tag="osb",
                                      bufs=4, name="osb")
                        if alt_pool and i % 2 == 1:
                            nc.scalar.copy(out=osb, in_=po_)
                        else:
                            nc.vector.tensor_copy(out=osb, in_=po_)
                        dma_eng = nc.sync if i % 2 == 0 else nc.gpsimd
                        dma_eng.dma_start(
                            out=outp[nch * 128:(nch + 1) * 128,
                                     qb * 512:(qb + 1) * 512],
                            in_=osb)
                    slots.append(mmstep)
                return slots

            tail_ops = []
            extra = []
            for nbp in range(NBP):
                for j in range(HPC):
                    # After an n-block boundary the previous block's last
                    # tail normalizes uta at steps 1/3; its out-proj may
                    # only start after that (step 5+ of the first pair).
                    estart = 5 if j == 0 else 0
                    tail_ops = emit_pair(nbp, j, tail_ops, extra, estart)
                if nbp < NBP - 1:
                    extra = extra + outproj_slots(nbp)
            # Tail: last pair's normalize, then the last n-block's
            # projection with PSUM buffers drawn from both pools so the
            # PE pipeline never starves on a single rotating buffer.
            for _, thunk in tail_ops:
                thunk()
            for thunk in extra:
                thunk()
            for thunk in outproj_slots(NBP - 1, alt_pool=True):
                thunk()

    lp.__exit__(None, None, None)
    nc.compile()
    return nc


_PROG_CACHE = {}


def _get_program(use_mask: bool):
    if use_mask not in _PROG_CACHE:
        _PROG_CACHE[use_mask] = build_program(use_mask)
    return _PROG_CACHE[use_mask]


def make_in_maps(y_prime, mask, Lam_x, Lam_y, Th_x, Th_y, use_mask):
    in_maps = []
    for c in range(NCORES):
        b = c // 4
        heads = [4 * (c % 4) + j for j in range(HPC)]
        m = {
            "yT": np.ascontiguousarray(y_prime[b].T).astype(np.float32),
            "lamx": np.ascontiguousarray(
                Lam_x[heads].transpose(1, 0, 2).reshape(P, HPC * D)),
            "lamy": np.ascontiguousarray(
                Lam_y[heads].transpose(1, 0, 2).reshape(P, HPC * D)),
            "thx": np.ascontiguousarray(
                Th_x[heads].transpose(1, 0, 2).reshape(P, HPC * D)),
            "thyT": np.ascontiguousarray(
                Th_y[heads].transpose(0, 2, 1).reshape(HPC * D, Q)),
        }
        if use_mask:
            m["mask"] = np.ascontiguousarray(mask).astype(np.float32)
        in_maps.append(m)
    return in_maps


def kernel(y_prime, mask, Lam_x, Lam_y, Th_x, Th_y, _trace=False):
    y_prime = np.asarray(y_prime, dtype=np.float32)
    mask = np.asarray(mask, dtype=np.float32)
    Lam_x = np.asarray(Lam_x, dtype=np.float32)
    Lam_y = np.asarray(Lam_y, dtype=np.float32)
    Th_x = np.asarray(Th_x, dtype=np.float32)
    Th_y = np.asarray(Th_y, dtype=np.float32)

    use_mask = bool(np.any(mask))
    nc = _get_program(use_mask)
    in_maps = make_in_maps(y_prime, mask, Lam_x, Lam_y, Th_x, Th_y, use_mask)
    r = run_bass_kernel_spmd(nc, in_maps, core_ids=list(range(NCORES)),
                             trace=_trace)
    out = np.zeros((B, N, Q), dtype=np.float32)
    for c in range(NCORES):
        out[c // 4] += r.results[c]["outp"]
    if _trace:
        kernel.last_results = r
    return out
